# revision 13
# baseline (speedup 1.0000x reference)
"""TRN2 Bass kernel for nn_Attention (cross-attention, Tq=2, Tk=5, B=16384, D=512).

Math reformulation (exact):
    logits = h . k~,  k~ = e @ W_qk,  W_qk = Wk @ Wq^T
    att = softmax(logits)
    out = h@Wd1 + ctx@W_vd,   ctx = att @ e,   W_vd = Wv @ Wd2
This removes the q and v projections entirely.

Sharding: pure data parallel over batch, 2048 per core x 8 cores.
Host marshals inputs/outputs to batch-major [B, T, D] for contiguous DMA.
On-chip compute in fp16 (PSUM accumulation fp32); final out fp32.
Main loop is software-pipelined (front: loads/transposes/k~; back: attention/out)
with a lag of 2 batch tiles so PE and DVE streams interleave across tiles.
"""

import contextlib

import numpy as np

import concourse.bass as bass
import concourse.mybir as mybir
import concourse.tile as tile
from concourse import bacc
from concourse.bass_utils import run_bass_kernel_spmd
from concourse.masks import make_identity

F32 = mybir.dt.float32
F16 = mybir.dt.float16
MUL = mybir.AluOpType.mult
ADD = mybir.AluOpType.add
BYP = mybir.AluOpType.bypass

TQ, TK, B, D = 2, 5, 16384, 512
NCORES = 8
BL = B // NCORES          # 2048 batch per core
P = 128                   # partition tile
NT = BL // P              # 16 batch tiles per core
DC = D // P               # 4 contraction chunks
LAG = 3                   # software-pipeline depth (front of t  ||  back of t-LAG)

_CACHED = {}


def build(reps=1, skip=()):
    nc = bacc.Bacc("TRN2", target_bir_lowering=False, debug=False)

    h_d = nc.dram_tensor("h", [BL, TQ, D], F16, kind="ExternalInput")
    e_d = nc.dram_tensor("enc", [BL, TK, D], F16, kind="ExternalInput")
    ht_d = nc.dram_tensor("hT", [NT, P, DC, TQ, P], F16, kind="ExternalInput")
    et_d = nc.dram_tensor("eT", [NT, P, DC, TK, P], F16, kind="ExternalInput")
    wq_d = nc.dram_tensor("Wq", [D, D], F32, kind="ExternalInput")
    wk_d = nc.dram_tensor("Wk", [D, D], F32, kind="ExternalInput")
    wv_d = nc.dram_tensor("Wv", [D, D], F32, kind="ExternalInput")
    wd_d = nc.dram_tensor("Wdown", [2 * D, D], F32, kind="ExternalInput")
    o_d = nc.dram_tensor("out", [BL, TQ, D], F32, kind="ExternalOutput")

    h_r = h_d.ap()
    e_r = e_d.ap()
    o_r = o_d.ap()

    with tile.TileContext(nc) as tc:
        with (
            tc.tile_pool(name="wgt", bufs=1) as wgt,
            tc.tile_pool(name="pre", bufs=1) as pre,
            tc.tile_pool(name="io", bufs=LAG + 2) as io,
            tc.tile_pool(name="work", bufs=LAG + 2) as work,
            tc.tile_pool(name="bwork", bufs=2) as bwork,
            tc.tile_pool(name="small", bufs=3) as small,
            tc.tile_pool(name="ps", bufs=2, space="PSUM") as ps,       # "pt": [P,8,P] 2bk x2
            tc.tile_pool(name="psk", bufs=2, space="PSUM") as psk,     # "pk": [P,512] 1bk x2
            tc.tile_pool(name="psb", bufs=2, space="PSUM") as psb,     # "po": 1bk x2
        ):
            ident = wgt.tile([P, P], F16)
            make_identity(nc, ident)

            # ---- load weights (cast fp32 -> fp16 during DMA) ----
            wq16 = pre.tile([P, DC, D], F16, tag="wq16")
            wk16 = pre.tile([P, DC, D], F16, tag="wk16")
            wv16 = pre.tile([P, DC, D], F16, tag="wv16")
            wd1 = wgt.tile([P, DC, D], F16, tag="wd1")
            wd2 = wgt.tile([P, DC, D], F16, tag="wd2")
            nc.gpsimd.dma_start(out=wq16, in_=wq_d.ap().rearrange("(c p) n -> p c n", p=P))
            nc.gpsimd.dma_start(out=wk16, in_=wk_d.ap().rearrange("(c p) n -> p c n", p=P))
            nc.gpsimd.dma_start(out=wv16, in_=wv_d.ap().rearrange("(c p) n -> p c n", p=P))
            nc.gpsimd.dma_start(out=wd1, in_=wd_d.ap()[:D].rearrange("(c p) n -> p c n", p=P))
            nc.gpsimd.dma_start(out=wd2, in_=wd_d.ap()[D:].rearrange("(c p) n -> p c n", p=P))

            # ---- transpose Wq, Wk, Wv via identity-matmul (2 waves of 8 blocks) ----
            def transpose_weight(w16, name):
                wT = pre.tile([P, DC, D], F16, tag=name, name=name)
                for w in range(2):
                    pt = ps.tile([P, 8, P], F32, tag="pt", name=f"pt_{name}{w}")
                    for gg in range(2):
                        g = w * 2 + gg
                        for a in range(DC):
                            nc.tensor.matmul(
                                pt[:, gg * 4 + a, :], w16[:, a, g * P:(g + 1) * P],
                                ident, start=True, stop=True)
                    nc.scalar.copy(
                        wT[:, w * 2:w * 2 + 2, :],
                        pt.rearrange("p (g a) b -> p g (a b)", g=2))
                return wT

            wqT = transpose_weight(wq16, "wqT")
            wkT = transpose_weight(wk16, "wkT")
            wvT = transpose_weight(wv16, "wvT")

            # ---- W_qk = Wk @ Wq^T ;  W_vd = Wv @ Wd2 ----
            wqk = wgt.tile([P, DC, D], F16, tag="wqk")
            wvd = wgt.tile([P, DC, D], F16, tag="wvd")
            for nm, (lhsT, rhs, dst) in {
                "q": (wkT, wqT, wqk), "v": (wvT, wd2, wvd)
            }.items():
                for ach in range(DC):
                    acc = psk.tile([P, D], F32, tag="pk", name=f"pk_{nm}{ach}")
                    for g in range(DC):
                        nc.tensor.matmul(
                            acc, lhsT[:, g, ach * P:(ach + 1) * P],
                            rhs[:, g, :], start=(g == 0), stop=(g == DC - 1))
                    nc.scalar.copy(dst[:, ach, :], acc)

            # ---- preload variant (for DMA-ablation benchmarking) ----
            pre_hn, pre_en = [], []
            if "dma" in skip:
                for t in range(NT):
                    phn = pre.tile([P, TQ, D], F16, tag=f"phn{t}", name=f"phn{t}")
                    pen = pre.tile([P, TK, D], F16, tag=f"pen{t}", name=f"pen{t}")
                    nc.gpsimd.dma_start(out=phn, in_=h_r[t * P:(t + 1) * P])
                    nc.gpsimd.dma_start(out=pen, in_=e_r[t * P:(t + 1) * P])
                    pre_hn.append(phn)
                    pre_en.append(pen)

            # ================= software-pipelined main loop =================
            def emit_front(t):
                bsl = slice(t * P, (t + 1) * P)
                if "dma" in skip:
                    hn, en = pre_hn[t], pre_en[t]
                else:
                    hn = io.tile([P, TQ, D], F16, tag="hn", name=f"hn{t}")
                    en = io.tile([P, TK, D], F16, tag="en", name=f"en{t}")
                    nc.sync.dma_start(out=hn, in_=h_r[bsl])
                    nc.sync.dma_start(out=en, in_=e_r[bsl])

                hT = work.tile([P, DC, TQ, P], F16, tag="hT", name=f"hT{t}")
                nc.sync.dma_start(out=hT, in_=ht_d.ap()[t])
                eT = work.tile([P, DC, TK, P], F16, tag="eT", name=f"eT{t}")
                nc.sync.dma_start(out=eT, in_=et_d.ap()[t])

                kn = work.tile([P, TK, D], F16, tag="kn", name=f"kn{t}")
                for j in range(TK):
                    acc = psk.tile([P, D], F32, tag="pk", name=f"pkk{t}_{j}")
                    for c in range(DC):
                        nc.tensor.matmul(
                            acc, eT[:, c, j, :], wqk[:, c, :],
                            start=(c == 0), stop=(c == DC - 1))
                    nc.scalar.copy(kn[:, j, :], acc)

                return dict(t=t, hn=hn, en=en, hT=hT, kn=kn)

            def emit_back(st):
                t, hn, en, hT, kn = st["t"], st["hn"], st["en"], st["hT"], st["kn"]
                bsl = slice(t * P, (t + 1) * P)

                if "attn" in skip:
                    ctx = bwork.tile([P, TQ, D], F16, tag="ctx", name=f"ctx{t}")
                    nc.vector.tensor_copy(ctx, kn[:, 0:2, :])
                else:
                    logits = small.tile([P, TQ, TK], F32, tag="logits", name=f"lg{t}")
                    pdump = small.tile([P, 1], F16, tag="pdump", name=f"pd{t}")
                    for i in range(TQ):
                        for j in range(TK):
                            nc.vector.scalar_tensor_tensor(
                                out=pdump.broadcast_to([P, D]),
                                in0=hn[:, i, :], scalar=1.0, in1=kn[:, j, :],
                                op0=BYP, op1=MUL,
                                accum_out=logits[:, i, j:j + 1])

                    nmx = small.tile([P, TQ], F32, tag="nmx", name=f"nm{t}")
                    pr = small.tile([P, TQ, TK], F32, tag="pr", name=f"pr{t}")
                    sm = small.tile([P, TQ], F32, tag="sm", name=f"sm{t}")
                    rs = small.tile([P, TQ], F32, tag="rs", name=f"rs{t}")
                    attw = small.tile([P, TQ, TK], F32, tag="attw", name=f"at{t}")
                    nc.vector.tensor_reduce(
                        out=nmx, in_=logits, axis=mybir.AxisListType.X,
                        op=mybir.AluOpType.max, negate=True)
                    for i in range(TQ):
                        nc.scalar.activation(
                            out=pr[:, i, :], in_=logits[:, i, :],
                            func=mybir.ActivationFunctionType.Exp,
                            bias=nmx[:, i:i + 1],
                            accum_out=sm[:, i:i + 1])
                    nc.vector.reciprocal(rs, sm)
                    for i in range(TQ):
                        nc.vector.tensor_scalar_mul(attw[:, i, :], pr[:, i, :], rs[:, i:i + 1])

                    ctx = bwork.tile([P, TQ, D], F16, tag="ctx", name=f"ctx{t}")
                    for i in range(TQ):
                        nc.vector.tensor_scalar_mul(ctx[:, i, :], en[:, 0, :], attw[:, i, 0:1])
                        for j in range(1, TK):
                            nc.vector.scalar_tensor_tensor(
                                out=ctx[:, i, :], in0=en[:, j, :],
                                scalar=attw[:, i, j:j + 1], in1=ctx[:, i, :],
                                op0=MUL, op1=ADD)

                cT = bwork.tile([P, DC, TQ, P], F16, tag="cT", name=f"cT{t}")
                ptc = ps.tile([P, 8, P], F32, tag="pt", name=f"ptc{t}")
                for i in range(TQ):
                    for c in range(DC):
                        nc.tensor.matmul(
                            ptc[:, i * DC + c, :], ctx[:, i, c * P:(c + 1) * P],
                            ident, start=True, stop=True)
                nc.vector.tensor_copy(cT, ptc.rearrange("p (i c) b -> p c i b", i=TQ))

                ob = io.tile([P, TQ, D], F32, tag="ob", name=f"ob{t}")
                for i in range(TQ):
                    po = psb.tile([P, D], F32, tag="po", name=f"po{t}_{i}")
                    for c in range(DC):
                        nc.tensor.matmul(po, hT[:, c, i, :], wd1[:, c, :],
                                         start=(c == 0), stop=False)
                    for c in range(DC):
                        nc.tensor.matmul(po, cT[:, c, i, :], wvd[:, c, :],
                                         start=False, stop=(c == DC - 1))
                    nc.vector.tensor_copy(ob[:, i, :], po)
                nc.gpsimd.dma_start(out=o_r[bsl], in_=ob)

            loop_cm = tc.For_i(0, reps, 1) if reps > 1 else contextlib.nullcontext()
            with loop_cm:
                pending = {}
                for tt in range(NT + LAG):
                    if tt < NT:
                        pending[tt] = emit_front(tt)
                    if tt >= LAG:
                        emit_back(pending.pop(tt - LAG))

    nc.compile()
    return nc


def kernel(h, enc_out, Wq, Wk, Wv, Wdown, _trace=False):
    h = np.ascontiguousarray(h, dtype=np.float32)
    enc_out = np.ascontiguousarray(enc_out, dtype=np.float32)
    Wq = np.ascontiguousarray(Wq, dtype=np.float32)
    Wk = np.ascontiguousarray(Wk, dtype=np.float32)
    Wv = np.ascontiguousarray(Wv, dtype=np.float32)
    Wdown = np.ascontiguousarray(Wdown, dtype=np.float32)

    if "nc" not in _CACHED:
        _CACHED["nc"] = build()
    nc = _CACHED["nc"]

    h16 = h.astype(np.float16)
    e16 = enc_out.astype(np.float16)
    h_bm = np.ascontiguousarray(h16.transpose(1, 0, 2))        # [B, TQ, D]
    e_bm = np.ascontiguousarray(e16.transpose(1, 0, 2))        # [B, TK, D]
    # block-transposed tiles: [core][t][p(d%128)][c][i][b]
    hT_bm = np.ascontiguousarray(
        h16.reshape(TQ, NCORES, NT, P, DC, P).transpose(1, 2, 5, 4, 0, 3))
    eT_bm = np.ascontiguousarray(
        e16.reshape(TK, NCORES, NT, P, DC, P).transpose(1, 2, 5, 4, 0, 3))
    in_maps = []
    for c in range(NCORES):
        sl = slice(c * BL, (c + 1) * BL)
        in_maps.append({
            "h": h_bm[sl],
            "enc": e_bm[sl],
            "hT": hT_bm[c],
            "eT": eT_bm[c],
            "Wq": Wq, "Wk": Wk, "Wv": Wv, "Wdown": Wdown,
        })

    res = run_bass_kernel_spmd(nc, in_maps, list(range(NCORES)), trace=_trace)
    out_bm = np.concatenate([r["out"] for r in res.results], axis=0)  # [B, TQ, D]
    out = np.ascontiguousarray(out_bm.transpose(1, 0, 2))
    if _trace:
        kernel.last_result = res
    return out.astype(np.float32)
